# revision 26
# baseline (speedup 1.0000x reference)
"""Trainium2 Bass kernel for a batched binary-tree (child-sum-ish) LSTM cell.

Computes, for N=8192 nodes (D = HD = 1024):
    z   = sigmoid([x_l x_r] @ W_z.T + b_z)
    x_t = z * x_l + (1-z) * x_r
    [x_i x_f x_o x_g] = x_t @ W_xin.T
    i   = sigmoid([h_l h_r c_l c_r] @ W_i.T  + b_i  + x_i)
    f_l = sigmoid([h_l h_r c_l c_r] @ W_fl.T + b_fl + x_f)
    f_r = sigmoid([h_l h_r c_l c_r] @ W_fr.T + b_fr + x_f)
    g   = tanh   ([h_l h_r]         @ W_g.T  + b_g  + x_g)
    c_t = f_l*c_l + f_r*c_r + i*g
    o   = sigmoid([h_l h_r c_t]     @ W_o.T  + b_o  + x_o)
    h_t = o * tanh(c_t)
returns (x_t, h_t, c_t).

Strategy: data-parallel over 8 NeuronCores (1024 rows each). All work is done
in transposed space (features on SBUF partitions, rows on the free dim) so no
on-device transposes are needed; the host pre-transposes activations and
pre-tiles weights. The x_in projection rides each gate's PSUM accumulation
(x_i in the i-gate, x_g in g, x_o in o; the shared x_f is computed once per
m-tile and DVE-added into both f-gate PSUMs). Matmuls run in bf16 with fp32
PSUM accumulate; elementwise math runs in fp32. Weight streaming uses the
sync HWDGE DMA ring, activation loads the SWDGE ring, and output stores the
scalar HWDGE ring, so the three flows never FIFO-block each other.
"""

import sys

if "/opt/trn_rl_repo" not in sys.path:
    sys.path.insert(0, "/opt/trn_rl_repo")

import numpy as np
import ml_dtypes

N_CORES = 8
N = 8192
D = 1024
P = 128
NL = N // N_CORES          # rows per core
KB = D // P                # 8 k-blocks per 1024-feature tensor
MT = D // P                # 8 output m-tiles per gate

# (name, K-tiles, bias index, act fn, [(rhs part, weight k-tile offset)],
#  xin key). The x_in projection x_t @ W_xin.T is precomputed per m-tile
# into SBUF a few m ahead (it needs no h/c data), then DVE-added into the
# gate PSUM — this gives the PE h/c-independent work to chew on while the
# h/c resident loads stream in.
_GATES_B = [
    ("i",  40, 1, "sig",  [("xtb", 32), ("hbl", 0), ("hbr", 8),
                           ("cbl", 16), ("cbr", 24)], None),
    ("fl", 32, 2, "sig",  [("hbl", 0), ("hbr", 8), ("cbl", 16), ("cbr", 24)], "xf"),
    ("fr", 32, 3, "sig",  [("hbl", 0), ("hbr", 8), ("cbl", 16), ("cbr", 24)], "xf"),
    ("g",  24, 4, "tanh", [("xtb", 16), ("hbl", 0), ("hbr", 8)], None),
]

_compiled = {}


def _build(R):
    """Build + compile the per-core Bass program. R = rows per slab."""
    import concourse.mybir as mybir
    import concourse.tile as tile
    from concourse import bacc

    F32 = mybir.dt.float32
    BF16 = mybir.dt.bfloat16
    SIG = mybir.ActivationFunctionType.Sigmoid
    TANH = mybir.ActivationFunctionType.Tanh

    assert NL % R == 0
    n_slabs = NL // R

    nc = bacc.Bacc("TRN2", target_bir_lowering=False, debug=False)

    def din(name, shape, dt):
        return nc.dram_tensor(name, shape, dt, kind="ExternalInput").ap()

    def dout(name, shape, dt):
        return nc.dram_tensor(name, shape, dt, kind="ExternalOutput").ap()

    # Transposed activations [D, NL]; b-prefix = bf16 (matmul), f = fp32.
    xb_l = din("xb_l", [D, NL], BF16)
    xb_r = din("xb_r", [D, NL], BF16)
    hb_l = din("hb_l", [D, NL], BF16)
    hb_r = din("hb_r", [D, NL], BF16)
    cb_l = din("cb_l", [D, NL], BF16)
    cb_r = din("cb_r", [D, NL], BF16)
    cf_l = din("cf_l", [D, NL], F32)
    cf_r = din("cf_r", [D, NL], F32)
    # Weights pre-tiled on host to [MT, P, Kt, P] (partition-major so each
    # per-partition DMA run is Kt*256B contiguous).
    wz = din("wz", [MT, P, 16, P], BF16)
    wi = din("wi", [MT, P, 40, P], BF16)
    wfl = din("wfl", [MT, P, 32, P], BF16)
    wfr = din("wfr", [MT, P, 32, P], BF16)
    wg = din("wg", [MT, P, 24, P], BF16)
    wo = din("wo", [MT, P, 32, P], BF16)
    wxf = din("wxf", [MT, P, 8, P], BF16)
    wmap = {"i": wi, "fl": wfl, "fr": wfr, "g": wg}
    bias = din("bias", [P, 6, MT], F32)

    xT_o = dout("xT_o", [D, NL], F32)
    hT_o = dout("hT_o", [D, NL], F32)
    cT_o = dout("cT_o", [D, NL], F32)

    def r3(ap):
        return ap.rearrange("(k p) n -> p k n", p=P)

    with tile.TileContext(nc) as tc:
        with (
            tc.tile_pool(name="acts", bufs=1) as apool,
            tc.tile_pool(name="w", bufs=4) as wpool,
            tc.tile_pool(name="gates", bufs=8) as gpool,
            tc.tile_pool(name="work", bufs=12) as wkpool,
            tc.tile_pool(name="ps", bufs=8, space="PSUM") as pspool,
            tc.tile_pool(name="cst", bufs=1) as cpool,
        ):
            bias_t = cpool.tile([P, 6, MT], F32, name="bias_t")
            nc.sync.dma_start(bias_t[:], bias[:])

            def load_xb(s):
                rs_ = slice(s * R, s * R + R)
                # Split the first load so the very first matmul's k-blocks
                # land sooner.
                xl = apool.tile([P, KB, R], BF16, tag="xbl", name="xbl")
                nc.gpsimd.dma_start(xl[:, :2, :], r3(xb_l)[:, :2, rs_])
                nc.gpsimd.dma_start(xl[:, 2:, :], r3(xb_l)[:, 2:, rs_])
                # x_r rides the scalar HWDGE ring, which is idle at slab
                # start, so both phase-A inputs stream concurrently.
                xr = apool.tile([P, KB, R], BF16, tag="xbr", name="xbr")
                nc.scalar.dma_start(xr[:], r3(xb_r)[:, :, rs_])
                return xl, xr

            next_xb = load_xb(0)
            for s in range(n_slabs):
                r0 = s * R
                rs = slice(r0, r0 + R)

                # Activations ride the SWDGE (gpsimd) DMA ring so they never
                # FIFO-block weight streaming on the sync (HWDGE) ring.
                def lda(name, dram, dt):
                    t = apool.tile([P, KB, R], dt, tag=name, name=name)
                    nc.gpsimd.dma_start(t[:], r3(dram)[:, :, rs])
                    return t

                xbl_t, xbr_t = next_xb
                xtb_t = apool.tile([P, KB, R], BF16, tag="xtb", name="xtb")
                ctb_t = apool.tile([P, KB, R], BF16, tag="ctb", name="ctb")
                tct_t = apool.tile([P, KB, R], F32, tag="tct", name="tct")

                # ---- Phase A: z gate + x_t ----
                for m in range(MT):
                    w_t = wpool.tile([P, 16, P], BF16, tag="w", name="wz_t")
                    nc.sync.dma_start(w_t[:], wz[m])
                    ps = pspool.tile([P, R], F32, tag="ps", name="ps_z")
                    for kt in range(16):
                        rhs = (xbl_t if kt < KB else xbr_t)[:, kt % KB, :]
                        nc.tensor.matmul(ps[:], w_t[:, kt, :], rhs,
                                         start=(kt == 0), stop=(kt == 15))
                    z_t = wkpool.tile([P, R], F32, tag="wk", name="z_t")
                    nc.scalar.activation(z_t[:], ps[:], SIG, bias=bias_t[:, 0, m, None])
                    d_t = wkpool.tile([P, R], F32, tag="wk", name="d_t")
                    nc.vector.tensor_sub(d_t[:], xbl_t[:, m, :], xbr_t[:, m, :])
                    xrf_m = wkpool.tile([P, R], F32, tag="wk", name="xrf_m")
                    nc.vector.tensor_copy(xrf_m[:], xbr_t[:, m, :])
                    nc.vector.tensor_mul(d_t[:], d_t[:], z_t[:])
                    xt_m = wkpool.tile([P, R], F32, tag="wk", name="xt_m")
                    nc.vector.tensor_add(xt_m[:], d_t[:], xrf_m[:])
                    nc.scalar.dma_start(r3(xT_o)[:, m, rs], xt_m[:])
                    # cast on ACT (idle here) — keeps the bf16 x_t that
                    # phase B waits on off the DVE critical path
                    nc.scalar.copy(xtb_t[:, m, :], xt_m[:])

                # Resident loads for phases B/C stream during phase-A compute.
                hbl_t = lda("hbl", hb_l, BF16)
                hbr_t = lda("hbr", hb_r, BF16)
                cbl_t = lda("cbl", cb_l, BF16)
                cbr_t = lda("cbr", cb_r, BF16)
                if s + 1 < n_slabs:
                    next_xb = load_xb(s + 1)
                parts = {"hbl": hbl_t, "hbr": hbr_t, "cbl": cbl_t,
                         "cbr": cbr_t, "xtb": xtb_t, "ctb": ctb_t}
                cfl_t = cfr_t = None

                # ---- Phase B: i, f_l, f_r, g gates + c_t ----
                for m in range(MT):
                    # x_f pre-activation is shared by f_l and f_r: compute it
                    # once per m-tile and DVE-add it into both gate PSUMs.
                    wxf_t = wpool.tile([P, KB, P], BF16, tag="w", name="wxf_t")
                    nc.sync.dma_start(wxf_t[:], wxf[m])
                    if m == 0:
                        # cf is first needed for c_t at the end of B m=0;
                        # loading here keeps it out of the phase-A window.
                        cfl_t = lda("cfl", cf_l, F32)
                        cfr_t = lda("cfr", cf_r, F32)
                    ps_xf = pspool.tile([P, R], F32, tag="ps", name="ps_xf")
                    for j in range(KB):
                        nc.tensor.matmul(ps_xf[:], wxf_t[:, j, :],
                                         xtb_t[:, j, :],
                                         start=(j == 0), stop=(j == KB - 1))
                    xfp_m = gpool.tile([P, R], F32, tag="gate", name="xfp_m")
                    nc.scalar.copy(xfp_m[:], ps_xf[:])
                    gt = {}
                    for (gname, Kt, b_idx, fn, rparts, xkey) in _GATES_B:
                        w_t = wpool.tile([P, Kt, P], BF16, tag="w",
                                         name=f"w_{gname}")
                        nc.sync.dma_start(w_t[:], wmap[gname][m])
                        ps = pspool.tile([P, R], F32, tag="ps",
                                         name=f"ps_{gname}")
                        n_done = 0
                        for (pname, koff) in rparts:
                            pt = parts[pname]
                            for j in range(KB):
                                nc.tensor.matmul(
                                    ps[:], w_t[:, koff + j, :], pt[:, j, :],
                                    start=(n_done == 0),
                                    stop=(n_done == Kt - 1))
                                n_done += 1
                        if xkey == "xf":
                            nc.vector.tensor_add(ps[:], ps[:], xfp_m[:])
                        g_t = gpool.tile([P, R], F32, tag="gate",
                                         name=f"g_{gname}")
                        nc.scalar.activation(
                            g_t[:], ps[:], SIG if fn == "sig" else TANH,
                            bias=bias_t[:, b_idx, m, None])
                        gt[gname] = g_t
                    ct_m = wkpool.tile([P, R], F32, tag="wk", name="ct_m")
                    nc.vector.tensor_mul(ct_m[:], gt["fl"][:], cfl_t[:, m, :])
                    t2 = wkpool.tile([P, R], F32, tag="wk", name="t2")
                    nc.vector.tensor_mul(t2[:], gt["fr"][:], cfr_t[:, m, :])
                    nc.vector.tensor_add(ct_m[:], ct_m[:], t2[:])
                    nc.vector.tensor_mul(t2[:], gt["i"][:], gt["g"][:])
                    nc.vector.tensor_add(ct_m[:], ct_m[:], t2[:])
                    nc.scalar.dma_start(r3(cT_o)[:, m, rs], ct_m[:])
                    nc.vector.tensor_copy(ctb_t[:, m, :], ct_m[:])
                    nc.scalar.activation(tct_t[:, m, :], ct_m[:], TANH)

                # ---- Phase C: o gate + h_t ----
                for m in range(MT):
                    w_t = wpool.tile([P, 32, P], BF16, tag="w", name="wo_t")
                    nc.sync.dma_start(w_t[:], wo[m])
                    ps = pspool.tile([P, R], F32, tag="ps", name="ps_o")
                    kt = 0
                    for pname in ["hbl", "hbr", "ctb", "xtb"]:
                        pt = parts[pname]
                        for j in range(KB):
                            nc.tensor.matmul(ps[:], w_t[:, kt, :], pt[:, j, :],
                                             start=(kt == 0), stop=(kt == 31))
                            kt += 1
                    o_t = wkpool.tile([P, R], F32, tag="wk", name="o_t")
                    nc.scalar.activation(o_t[:], ps[:], SIG, bias=bias_t[:, 5, m, None])
                    ht_m = wkpool.tile([P, R], F32, tag="wk", name="ht_m")
                    nc.vector.tensor_mul(ht_m[:], o_t[:], tct_t[:, m, :])
                    nc.scalar.dma_start(r3(hT_o)[:, m, rs], ht_m[:])

    nc.compile()
    return nc


def _get_compiled(R=512):
    if R not in _compiled:
        _compiled[R] = _build(R)
    return _compiled[R]


def _prep_weight(w_km):
    """[K, D] (K-major stack of W.T blocks) -> [MT, P, Kt, P] bf16."""
    K = w_km.shape[0]
    kt = K // P
    w = w_km.reshape(kt, P, MT, P)          # [kt, p, m, f]
    w = np.ascontiguousarray(w.transpose(2, 1, 0, 3))  # [m, p, kt, f]
    return w.astype(ml_dtypes.bfloat16)


def _host_prep(inp):
    """Transpose/stack/cast everything the device program wants."""
    f32 = np.float32
    t = {k: np.ascontiguousarray(np.asarray(inp[k], dtype=f32).T)
         for k in ("x_l", "x_r", "h_l", "h_r", "c_l", "c_r")}
    bf = {k: v.astype(ml_dtypes.bfloat16) for k, v in t.items()}

    W_i = np.asarray(inp["W_i"], f32)
    W_fl = np.asarray(inp["W_fl"], f32)
    W_fr = np.asarray(inp["W_fr"], f32)
    W_xin = np.asarray(inp["W_xin"], f32)
    W_o = np.asarray(inp["W_o"], f32)
    W_z = np.asarray(inp["W_z"], f32)
    W_g = np.asarray(inp["W_g"], f32)

    x_i = W_xin[0 * D:1 * D].T    # [D, D] blocks of W_xin.T
    x_f = W_xin[1 * D:2 * D].T
    x_o = W_xin[2 * D:3 * D].T
    x_g = W_xin[3 * D:4 * D].T

    weights = {
        "wz": _prep_weight(np.ascontiguousarray(W_z.T)),
        "wi": _prep_weight(np.concatenate([W_i.T, x_i], axis=0)),
        "wfl": _prep_weight(np.ascontiguousarray(W_fl.T)),
        "wfr": _prep_weight(np.ascontiguousarray(W_fr.T)),
        "wg": _prep_weight(np.concatenate([W_g.T, x_g], axis=0)),
        "wo": _prep_weight(np.concatenate([W_o.T, x_o], axis=0)),
        "wxf": _prep_weight(x_f),
    }

    b = np.stack([np.asarray(inp[k], f32) for k in
                  ("b_z", "b_i", "b_fl", "b_fr", "b_g", "b_o")])  # [6, D]
    bias = np.ascontiguousarray(b.reshape(6, MT, P).transpose(2, 0, 1))

    in_maps = []
    for c in range(N_CORES):
        cs = slice(c * NL, (c + 1) * NL)
        m = {
            "xb_l": np.ascontiguousarray(bf["x_l"][:, cs]),
            "xb_r": np.ascontiguousarray(bf["x_r"][:, cs]),
            "hb_l": np.ascontiguousarray(bf["h_l"][:, cs]),
            "hb_r": np.ascontiguousarray(bf["h_r"][:, cs]),
            "cb_l": np.ascontiguousarray(bf["c_l"][:, cs]),
            "cb_r": np.ascontiguousarray(bf["c_r"][:, cs]),
            "cf_l": np.ascontiguousarray(t["c_l"][:, cs]),
            "cf_r": np.ascontiguousarray(t["c_r"][:, cs]),
            "bias": bias,
        }
        m.update(weights)
        in_maps.append(m)
    return in_maps


def run(inputs, R=512, trace=False, trace_kwargs=None):
    """Run on 8 cores; returns (results, BassKernelResults)."""
    from concourse.bass_utils import run_bass_kernel_spmd

    if trace:
        try:
            from hookfix import install_ntff_hook
            install_ntff_hook()
        except Exception:
            pass
    nc = _get_compiled(R)
    in_maps = _host_prep(inputs)
    res = run_bass_kernel_spmd(nc, in_maps, core_ids=list(range(N_CORES)),
                               trace=trace, **(trace_kwargs or {}))
    xT = np.concatenate([res.results[c]["xT_o"] for c in range(N_CORES)], axis=1)
    hT = np.concatenate([res.results[c]["hT_o"] for c in range(N_CORES)], axis=1)
    cT = np.concatenate([res.results[c]["cT_o"] for c in range(N_CORES)], axis=1)
    x_t = np.ascontiguousarray(xT.T)
    h_t = np.ascontiguousarray(hT.T)
    c_t = np.ascontiguousarray(cT.T)
    return (x_t, h_t, c_t), res


def kernel(**inputs):
    out, _ = run(inputs)
    return out


# revision 27
# speedup vs baseline: 1.0209x; 1.0209x over previous
"""Trainium2 Bass kernel for a batched binary-tree (child-sum-ish) LSTM cell.

Computes, for N=8192 nodes (D = HD = 1024):
    z   = sigmoid([x_l x_r] @ W_z.T + b_z)
    x_t = z * x_l + (1-z) * x_r
    [x_i x_f x_o x_g] = x_t @ W_xin.T
    i   = sigmoid([h_l h_r c_l c_r] @ W_i.T  + b_i  + x_i)
    f_l = sigmoid([h_l h_r c_l c_r] @ W_fl.T + b_fl + x_f)
    f_r = sigmoid([h_l h_r c_l c_r] @ W_fr.T + b_fr + x_f)
    g   = tanh   ([h_l h_r]         @ W_g.T  + b_g  + x_g)
    c_t = f_l*c_l + f_r*c_r + i*g
    o   = sigmoid([h_l h_r c_t]     @ W_o.T  + b_o  + x_o)
    h_t = o * tanh(c_t)
returns (x_t, h_t, c_t).

Strategy: data-parallel over 8 NeuronCores (1024 rows each). All work is done
in transposed space (features on SBUF partitions, rows on the free dim) so no
on-device transposes are needed; the host pre-transposes activations and
pre-tiles weights. The x_in projection rides each gate's PSUM accumulation
(x_i in the i-gate, x_g in g, x_o in o; the shared x_f is computed once per
m-tile and DVE-added into both f-gate PSUMs). Matmuls run in bf16 with fp32
PSUM accumulate; elementwise math runs in fp32. Weight streaming uses the
sync HWDGE DMA ring, activation loads the SWDGE ring, and output stores the
scalar HWDGE ring, so the three flows never FIFO-block each other.
"""

import sys

if "/opt/trn_rl_repo" not in sys.path:
    sys.path.insert(0, "/opt/trn_rl_repo")

import numpy as np
import ml_dtypes

N_CORES = 8
N = 8192
D = 1024
P = 128
NL = N // N_CORES          # rows per core
KB = D // P                # 8 k-blocks per 1024-feature tensor
MT = D // P                # 8 output m-tiles per gate

# (name, K-tiles, bias index, act fn, [(rhs part, weight k-tile offset)],
#  xin key). The x_in projection x_t @ W_xin.T is precomputed per m-tile
# into SBUF a few m ahead (it needs no h/c data), then DVE-added into the
# gate PSUM — this gives the PE h/c-independent work to chew on while the
# h/c resident loads stream in.
_GATES_B = [
    ("i",  40, 1, "sig",  [("xtb", 32), ("hbl", 0), ("hbr", 8),
                           ("cbl", 16), ("cbr", 24)], None),
    ("fl", 32, 2, "sig",  [("hbl", 0), ("hbr", 8), ("cbl", 16), ("cbr", 24)], "xf"),
    ("fr", 32, 3, "sig",  [("hbl", 0), ("hbr", 8), ("cbl", 16), ("cbr", 24)], "xf"),
    ("g",  24, 4, "tanh", [("xtb", 16), ("hbl", 0), ("hbr", 8)], None),
]

_compiled = {}


def _build(R):
    """Build + compile the per-core Bass program. R = rows per slab."""
    import concourse.mybir as mybir
    import concourse.tile as tile
    from concourse import bacc

    F32 = mybir.dt.float32
    BF16 = mybir.dt.bfloat16
    SIG = mybir.ActivationFunctionType.Sigmoid
    TANH = mybir.ActivationFunctionType.Tanh

    assert NL % R == 0
    n_slabs = NL // R

    nc = bacc.Bacc("TRN2", target_bir_lowering=False, debug=False)

    def din(name, shape, dt):
        return nc.dram_tensor(name, shape, dt, kind="ExternalInput").ap()

    def dout(name, shape, dt):
        return nc.dram_tensor(name, shape, dt, kind="ExternalOutput").ap()

    # Transposed activations [D, NL]; b-prefix = bf16 (matmul), f = fp32.
    xb_l = din("xb_l", [D, NL], BF16)
    xb_r = din("xb_r", [D, NL], BF16)
    hb_l = din("hb_l", [D, NL], BF16)
    hb_r = din("hb_r", [D, NL], BF16)
    cb_l = din("cb_l", [D, NL], BF16)
    cb_r = din("cb_r", [D, NL], BF16)
    cf_l = din("cf_l", [D, NL], F32)
    cf_r = din("cf_r", [D, NL], F32)
    # Weights pre-tiled on host to [MT, P, Kt, P] (partition-major so each
    # per-partition DMA run is Kt*256B contiguous).
    wz = din("wz", [MT, P, 16, P], BF16)
    wi = din("wi", [MT, P, 40, P], BF16)
    wfl = din("wfl", [MT, P, 32, P], BF16)
    wfr = din("wfr", [MT, P, 32, P], BF16)
    wg = din("wg", [MT, P, 24, P], BF16)
    wo = din("wo", [MT, P, 32, P], BF16)
    wxf = din("wxf", [MT, P, 8, P], BF16)
    wmap = {"i": wi, "fl": wfl, "fr": wfr, "g": wg}
    bias = din("bias", [P, 6, MT], F32)

    xT_o = dout("xT_o", [D, NL], F32)
    hT_o = dout("hT_o", [D, NL], F32)
    cT_o = dout("cT_o", [D, NL], F32)

    def r3(ap):
        return ap.rearrange("(k p) n -> p k n", p=P)

    with tile.TileContext(nc) as tc:
        with (
            tc.tile_pool(name="acts", bufs=1) as apool,
            tc.tile_pool(name="w", bufs=4) as wpool,
            tc.tile_pool(name="gates", bufs=8) as gpool,
            tc.tile_pool(name="work", bufs=12) as wkpool,
            tc.tile_pool(name="ps", bufs=8, space="PSUM") as pspool,
            tc.tile_pool(name="cst", bufs=1) as cpool,
        ):
            bias_t = cpool.tile([P, 6, MT], F32, name="bias_t")
            nc.sync.dma_start(bias_t[:], bias[:])

            def load_xb(s):
                rs_ = slice(s * R, s * R + R)
                # Split the first load so the very first matmul's k-blocks
                # land sooner.
                xl = apool.tile([P, KB, R], BF16, tag="xbl", name="xbl")
                nc.gpsimd.dma_start(xl[:, :2, :], r3(xb_l)[:, :2, rs_])
                nc.gpsimd.dma_start(xl[:, 2:, :], r3(xb_l)[:, 2:, rs_])
                # x_r rides the scalar HWDGE ring, which is idle at slab
                # start, so both phase-A inputs stream concurrently.
                xr = apool.tile([P, KB, R], BF16, tag="xbr", name="xbr")
                nc.scalar.dma_start(xr[:], r3(xb_r)[:, :, rs_])
                return xl, xr

            next_xb = load_xb(0)
            for s in range(n_slabs):
                r0 = s * R
                rs = slice(r0, r0 + R)

                # Activations ride the SWDGE (gpsimd) DMA ring so they never
                # FIFO-block weight streaming on the sync (HWDGE) ring.
                def lda(name, dram, dt):
                    t = apool.tile([P, KB, R], dt, tag=name, name=name)
                    nc.gpsimd.dma_start(t[:], r3(dram)[:, :, rs])
                    return t

                xbl_t, xbr_t = next_xb
                xtb_t = apool.tile([P, KB, R], BF16, tag="xtb", name="xtb")
                ctb_t = apool.tile([P, KB, R], BF16, tag="ctb", name="ctb")
                tct_t = apool.tile([P, KB, R], F32, tag="tct", name="tct")

                # ---- Phase A: z gate + x_t ----
                for m in range(MT):
                    w_t = wpool.tile([P, 16, P], BF16, tag="w", name="wz_t")
                    nc.sync.dma_start(w_t[:], wz[m])
                    ps = pspool.tile([P, R], F32, tag="ps", name="ps_z")
                    for kt in range(16):
                        rhs = (xbl_t if kt < KB else xbr_t)[:, kt % KB, :]
                        nc.tensor.matmul(ps[:], w_t[:, kt, :], rhs,
                                         start=(kt == 0), stop=(kt == 15))
                    z_t = wkpool.tile([P, R], F32, tag="wk", name="z_t")
                    nc.scalar.activation(z_t[:], ps[:], SIG, bias=bias_t[:, 0, m, None])
                    d_t = wkpool.tile([P, R], F32, tag="wk", name="d_t")
                    nc.vector.tensor_sub(d_t[:], xbl_t[:, m, :], xbr_t[:, m, :])
                    xrf_m = wkpool.tile([P, R], F32, tag="wk", name="xrf_m")
                    nc.vector.tensor_copy(xrf_m[:], xbr_t[:, m, :])
                    nc.vector.tensor_mul(d_t[:], d_t[:], z_t[:])
                    xt_m = wkpool.tile([P, R], F32, tag="wk", name="xt_m")
                    nc.vector.tensor_add(xt_m[:], d_t[:], xrf_m[:])
                    nc.scalar.dma_start(r3(xT_o)[:, m, rs], xt_m[:])
                    nc.vector.tensor_copy(xtb_t[:, m, :], xt_m[:])

                # Resident loads for phases B/C stream during phase-A compute.
                hbl_t = lda("hbl", hb_l, BF16)
                hbr_t = lda("hbr", hb_r, BF16)
                cbl_t = lda("cbl", cb_l, BF16)
                cbr_t = lda("cbr", cb_r, BF16)
                if s + 1 < n_slabs:
                    next_xb = load_xb(s + 1)
                parts = {"hbl": hbl_t, "hbr": hbr_t, "cbl": cbl_t,
                         "cbr": cbr_t, "xtb": xtb_t, "ctb": ctb_t}
                cfl_t = cfr_t = None

                # ---- Phase B: i, f_l, f_r, g gates + c_t ----
                for m in range(MT):
                    # x_f pre-activation is shared by f_l and f_r: compute it
                    # once per m-tile and DVE-add it into both gate PSUMs.
                    wxf_t = wpool.tile([P, KB, P], BF16, tag="w", name="wxf_t")
                    nc.sync.dma_start(wxf_t[:], wxf[m])
                    if m == 0:
                        # cf is first needed for c_t at the end of B m=0;
                        # loading here keeps it out of the phase-A window.
                        cfl_t = lda("cfl", cf_l, F32)
                        cfr_t = lda("cfr", cf_r, F32)
                    ps_xf = pspool.tile([P, R], F32, tag="ps", name="ps_xf")
                    for j in range(KB):
                        nc.tensor.matmul(ps_xf[:], wxf_t[:, j, :],
                                         xtb_t[:, j, :],
                                         start=(j == 0), stop=(j == KB - 1))
                    xfp_m = gpool.tile([P, R], F32, tag="gate", name="xfp_m")
                    nc.scalar.copy(xfp_m[:], ps_xf[:])
                    gt = {}
                    for (gname, Kt, b_idx, fn, rparts, xkey) in _GATES_B:
                        w_t = wpool.tile([P, Kt, P], BF16, tag="w",
                                         name=f"w_{gname}")
                        nc.sync.dma_start(w_t[:], wmap[gname][m])
                        ps = pspool.tile([P, R], F32, tag="ps",
                                         name=f"ps_{gname}")
                        n_done = 0
                        for (pname, koff) in rparts:
                            pt = parts[pname]
                            for j in range(KB):
                                nc.tensor.matmul(
                                    ps[:], w_t[:, koff + j, :], pt[:, j, :],
                                    start=(n_done == 0),
                                    stop=(n_done == Kt - 1))
                                n_done += 1
                        if xkey == "xf":
                            nc.vector.tensor_add(ps[:], ps[:], xfp_m[:])
                        g_t = gpool.tile([P, R], F32, tag="gate",
                                         name=f"g_{gname}")
                        nc.scalar.activation(
                            g_t[:], ps[:], SIG if fn == "sig" else TANH,
                            bias=bias_t[:, b_idx, m, None])
                        gt[gname] = g_t
                    ct_m = wkpool.tile([P, R], F32, tag="wk", name="ct_m")
                    nc.vector.tensor_mul(ct_m[:], gt["fl"][:], cfl_t[:, m, :])
                    t2 = wkpool.tile([P, R], F32, tag="wk", name="t2")
                    nc.vector.tensor_mul(t2[:], gt["fr"][:], cfr_t[:, m, :])
                    nc.vector.tensor_add(ct_m[:], ct_m[:], t2[:])
                    nc.vector.tensor_mul(t2[:], gt["i"][:], gt["g"][:])
                    nc.vector.tensor_add(ct_m[:], ct_m[:], t2[:])
                    nc.scalar.dma_start(r3(cT_o)[:, m, rs], ct_m[:])
                    nc.vector.tensor_copy(ctb_t[:, m, :], ct_m[:])
                    nc.scalar.activation(tct_t[:, m, :], ct_m[:], TANH)

                # ---- Phase C: o gate + h_t ----
                for m in range(MT):
                    w_t = wpool.tile([P, 32, P], BF16, tag="w", name="wo_t")
                    nc.sync.dma_start(w_t[:], wo[m])
                    ps = pspool.tile([P, R], F32, tag="ps", name="ps_o")
                    kt = 0
                    for pname in ["hbl", "hbr", "ctb", "xtb"]:
                        pt = parts[pname]
                        for j in range(KB):
                            nc.tensor.matmul(ps[:], w_t[:, kt, :], pt[:, j, :],
                                             start=(kt == 0), stop=(kt == 31))
                            kt += 1
                    o_t = wkpool.tile([P, R], F32, tag="wk", name="o_t")
                    nc.scalar.activation(o_t[:], ps[:], SIG, bias=bias_t[:, 5, m, None])
                    ht_m = wkpool.tile([P, R], F32, tag="wk", name="ht_m")
                    nc.vector.tensor_mul(ht_m[:], o_t[:], tct_t[:, m, :])
                    nc.scalar.dma_start(r3(hT_o)[:, m, rs], ht_m[:])

    nc.compile()
    return nc


def _get_compiled(R=512):
    if R not in _compiled:
        _compiled[R] = _build(R)
    return _compiled[R]


def _prep_weight(w_km):
    """[K, D] (K-major stack of W.T blocks) -> [MT, P, Kt, P] bf16."""
    K = w_km.shape[0]
    kt = K // P
    w = w_km.reshape(kt, P, MT, P)          # [kt, p, m, f]
    w = np.ascontiguousarray(w.transpose(2, 1, 0, 3))  # [m, p, kt, f]
    return w.astype(ml_dtypes.bfloat16)


def _host_prep(inp):
    """Transpose/stack/cast everything the device program wants."""
    f32 = np.float32
    t = {k: np.ascontiguousarray(np.asarray(inp[k], dtype=f32).T)
         for k in ("x_l", "x_r", "h_l", "h_r", "c_l", "c_r")}
    bf = {k: v.astype(ml_dtypes.bfloat16) for k, v in t.items()}

    W_i = np.asarray(inp["W_i"], f32)
    W_fl = np.asarray(inp["W_fl"], f32)
    W_fr = np.asarray(inp["W_fr"], f32)
    W_xin = np.asarray(inp["W_xin"], f32)
    W_o = np.asarray(inp["W_o"], f32)
    W_z = np.asarray(inp["W_z"], f32)
    W_g = np.asarray(inp["W_g"], f32)

    x_i = W_xin[0 * D:1 * D].T    # [D, D] blocks of W_xin.T
    x_f = W_xin[1 * D:2 * D].T
    x_o = W_xin[2 * D:3 * D].T
    x_g = W_xin[3 * D:4 * D].T

    weights = {
        "wz": _prep_weight(np.ascontiguousarray(W_z.T)),
        "wi": _prep_weight(np.concatenate([W_i.T, x_i], axis=0)),
        "wfl": _prep_weight(np.ascontiguousarray(W_fl.T)),
        "wfr": _prep_weight(np.ascontiguousarray(W_fr.T)),
        "wg": _prep_weight(np.concatenate([W_g.T, x_g], axis=0)),
        "wo": _prep_weight(np.concatenate([W_o.T, x_o], axis=0)),
        "wxf": _prep_weight(x_f),
    }

    b = np.stack([np.asarray(inp[k], f32) for k in
                  ("b_z", "b_i", "b_fl", "b_fr", "b_g", "b_o")])  # [6, D]
    bias = np.ascontiguousarray(b.reshape(6, MT, P).transpose(2, 0, 1))

    in_maps = []
    for c in range(N_CORES):
        cs = slice(c * NL, (c + 1) * NL)
        m = {
            "xb_l": np.ascontiguousarray(bf["x_l"][:, cs]),
            "xb_r": np.ascontiguousarray(bf["x_r"][:, cs]),
            "hb_l": np.ascontiguousarray(bf["h_l"][:, cs]),
            "hb_r": np.ascontiguousarray(bf["h_r"][:, cs]),
            "cb_l": np.ascontiguousarray(bf["c_l"][:, cs]),
            "cb_r": np.ascontiguousarray(bf["c_r"][:, cs]),
            "cf_l": np.ascontiguousarray(t["c_l"][:, cs]),
            "cf_r": np.ascontiguousarray(t["c_r"][:, cs]),
            "bias": bias,
        }
        m.update(weights)
        in_maps.append(m)
    return in_maps


def run(inputs, R=512, trace=False, trace_kwargs=None):
    """Run on 8 cores; returns (results, BassKernelResults)."""
    from concourse.bass_utils import run_bass_kernel_spmd

    if trace:
        try:
            from hookfix import install_ntff_hook
            install_ntff_hook()
        except Exception:
            pass
    nc = _get_compiled(R)
    in_maps = _host_prep(inputs)
    res = run_bass_kernel_spmd(nc, in_maps, core_ids=list(range(N_CORES)),
                               trace=trace, **(trace_kwargs or {}))
    xT = np.concatenate([res.results[c]["xT_o"] for c in range(N_CORES)], axis=1)
    hT = np.concatenate([res.results[c]["hT_o"] for c in range(N_CORES)], axis=1)
    cT = np.concatenate([res.results[c]["cT_o"] for c in range(N_CORES)], axis=1)
    x_t = np.ascontiguousarray(xT.T)
    h_t = np.ascontiguousarray(hT.T)
    c_t = np.ascontiguousarray(cT.T)
    return (x_t, h_t, c_t), res


def kernel(**inputs):
    out, _ = run(inputs)
    return out
